# revision 21
# baseline (speedup 1.0000x reference)
"""HAWQ tiny classifier on 8 TRN2 cores — pure data parallel, v3.

Per core: batch shard [2048, 2000].
Host precomputes: gmax=max|sig|, w1s/w2s weight scales, int weights (bf16),
b1 integer quantization. Device pipeline (tile-granular, no DRAM bounce):
  q   = round(sig * 15/gmax)          bf16 in SBUF
  qT  = PE-transpose of q (128x128 blocks via identity matmuls -> PSUM),
        copied PSUM->SBUF per half-tile (Scalar), quarter-batched GEMM1
  r   = relu(a1 + b1int); per-quarter feature stats [sum, sumsq, max]
  ONE AllGather of [100,3] stats (warmup collective absorbs launch skew)
  BN folded with approx stats (rounding-variance corrected: var += s2^2/12)
  y   = round(r*127/maxr) * (abn*s2*w2s);  z = w2f.T @ y + zbias  (f32 PE,
        8 batch-chunks written at psum partition offsets -> [16,256] layout)
  zr = relu(z); AllGather #2 of maxz; out = round(zr*127/maxz)*maxz/127
Output written [16, 256] per core; host reshapes/concats.
Rounding uses the f32 magic-number trick (+1.5*2^23 = round-to-nearest-even).
"""

import os
import sys

for p in ("/opt/trn_rl_repo", "/opt/trn_rl_repo/concourse"):
    if p not in sys.path:
        sys.path.insert(0, p)

import numpy as np
import ml_dtypes

import concourse.bass as bass
import concourse.bacc as bacc
import concourse.tile as tile
import concourse.mybir as mybir
from concourse import bass_utils
from concourse._compat import with_exitstack

F32 = mybir.dt.float32
BF16 = mybir.dt.bfloat16

BATCH, D_IN, HID, OUT = 16384, 2000, 100, 2
NCORES = 8
SHARD = BATCH // NCORES          # 2048 rows per core
NT = SHARD // 128                # 16 batch tiles per core
NQ = 5                           # phase-B chunks
QT = NT // NQ                    # 4 tiles per quarter
QROWS = SHARD // NQ              # 512 rows per quarter
KP = 2048                        # padded contraction dim (2000 -> 16*128)
NK = KP // 128                   # 16 k-chunks
MAGIC = 12582912.0               # 1.5 * 2**23
BN_EPS = 1e-5
NZ = 4                           # batch chunks for layer-2 output
ZC = SHARD // NZ                 # 512 cols per z chunk

_CACHE = {}


def _build(w1s: float, w2s: float, gmax: float):
    nc = bacc.Bacc(
        "TRN2",
        target_bir_lowering=False,
        debug=False,
        enable_asserts=False,
        num_devices=NCORES,
    )

    sig = nc.dram_tensor("sig", [SHARD, D_IN], F32, kind="ExternalInput")
    w1t = nc.dram_tensor("w1t", [KP, HID], BF16, kind="ExternalInput")
    w2t = nc.dram_tensor("w2t", [HID, OUT], F32, kind="ExternalInput")
    b1i = nc.dram_tensor("b1i", [HID, 1], F32, kind="ExternalInput")
    b2 = nc.dram_tensor("b2", [1, OUT], F32, kind="ExternalInput")
    gam = nc.dram_tensor("gamma", [HID, 1], F32, kind="ExternalInput")
    bet = nc.dram_tensor("bet", [HID, 1], F32, kind="ExternalInput")
    ident = nc.dram_tensor("ident", [128, 128], F32, kind="ExternalInput")
    out = nc.dram_tensor("out", [OUT, SHARD], F32, kind="ExternalOutput")
    rg = [list(range(NCORES))]

    with tile.TileContext(nc) as tc:
        _kern(tc, nc, sig, w1t, w2t, b1i, b2, gam, bet, ident, out,
              rg, w1s, w2s, gmax)
    nc.compile()
    return nc


@with_exitstack
def _kern(ctx, tc, nc, sig, w1t, w2t, b1i, b2, gam, bet, ident, out, rg,
          w1s, w2s, gmax):
    S1 = w1s * gmax / 15.0           # scale of r (real = r*S1)

    sigp = ctx.enter_context(tc.tile_pool(name="sigp", bufs=3))
    tmpp = ctx.enter_context(tc.tile_pool(name="tmpp", bufs=2))
    qp = ctx.enter_context(tc.tile_pool(name="qp", bufs=3))
    qtsp = ctx.enter_context(tc.tile_pool(name="qtsp", bufs=2))
    wp = ctx.enter_context(tc.tile_pool(name="wp", bufs=1))
    hp = ctx.enter_context(tc.tile_pool(name="hp", bufs=1))      # [HID,2048]
    sp = ctx.enter_context(tc.tile_pool(name="sp", bufs=1))      # small stats
    zp = ctx.enter_context(tc.tile_pool(name="zp", bufs=2))      # [16,256]
    psb = ctx.enter_context(tc.tile_pool(name="psb", bufs=1, space="PSUM"))
    ptr = ctx.enter_context(tc.tile_pool(name="ptr", bufs=1, space="PSUM"))
    pss = ctx.enter_context(tc.tile_pool(name="pss", bufs=2, space="PSUM"))
    dcc = ctx.enter_context(tc.tile_pool(name="dcc", bufs=1, space="DRAM"))

    # ---- prologue: small loads (SP queue) ----
    id_sb = sp.tile([128, 128], F32, tag="ident")
    nc.sync.dma_start(id_sb[:], ident[:, :])
    idb = sp.tile([128, 128], BF16, tag="idb")
    nc.vector.tensor_copy(idb[:], id_sb[:])
    one1 = sp.tile([1, 1], F32, tag="one1")
    nc.vector.memset(one1[:], 1.0)
    b1_sb = sp.tile([HID, 1], F32, tag="b1i")
    nc.sync.dma_start(b1_sb[:], b1i[:, :])
    b2_sb = sp.tile([1, OUT], F32, tag="b2")
    nc.sync.dma_start(b2_sb[:], b2[:, :])
    gam_sb = sp.tile([HID, 1], F32, tag="gam")
    nc.sync.dma_start(gam_sb[:], gam[:, :])
    bet_sb = sp.tile([HID, 1], F32, tag="bet")
    nc.sync.dma_start(bet_sb[:], bet[:, :])
    w2f = sp.tile([HID, OUT], F32, tag="w2f")
    nc.sync.dma_start(w2f[:], w2t[:, :])
    w1c = []
    for k in range(NK):
        wt = wp.tile([128, HID], BF16, tag=f"w1_{k}")
        nc.sync.dma_start(wt[:], w1t[k * 128:(k + 1) * 128, :])
        w1c.append(wt)
    # warmup collective: absorbs cross-core launch skew off the critical path
    wdin = dcc.tile([1, 1], F32, tag="wu_i")
    wdout = dcc.tile([NCORES, 1], F32, tag="wu_o")
    nc.sync.dma_start(wdin[:], one1[:])
    nc.gpsimd.collective_compute(
        "AllGather", mybir.AluOpType.bypass, replica_groups=rg,
        ins=[wdin.opt()], outs=[wdout.opt()])

    # ---------- helpers ----------
    def bcast(scal, n, val, tag):
        """[n,1] f32 = val * scal (scal is [1,1])."""
        r = sp.tile([n, 1], F32, tag=tag)
        nc.gpsimd.partition_broadcast(r[:], scal[:])
        if val != 1.0:
            nc.vector.tensor_scalar_mul(r[:], r[:], float(val))
        return r

    # ---------- phase B+C: load, quantize, PE-transpose, GEMM1 ----------
    qsc = 15.0 / gmax
    ps_a1 = psb.tile([HID, SHARD], F32, tag="big")
    r = hp.tile([HID, SHARD], F32, tag="h")
    sqh = hp.tile([HID, SHARD], F32, tag="sqh")
    st_s = sp.tile([HID, NQ], F32, tag="st_s")
    st_q = sp.tile([HID, NQ], F32, tag="st_q")
    st_m = sp.tile([HID, NQ], F32, tag="st_m")

    QSIZES = (4, 4, 4, 3, 1)
    qbase = 0
    for q in range(NQ):
        qn = QSIZES[q]
        qts = qtsp.tile([128, NK * qn * 128], BF16, tag="qts")
        # layout [p, (t k c)]: tile-major -> contiguous PSUM->SBUF copies
        qtsv = qts[:].rearrange("p (t k c) -> p t k c", t=qn, k=NK)
        for tq in range(qn):
            t = qbase + tq
            st = sigp.tile([128, D_IN], F32, tag="sig")
            eng = nc.scalar if t % 2 == 0 else nc.sync
            eng.dma_start(st[:], sig[t * 128:(t + 1) * 128, :])
            v1 = tmpp.tile([128, D_IN], F32, tag="v1")
            if t % 2 == 0:
                nc.scalar.activation(v1[:], st[:],
                                     mybir.ActivationFunctionType.Copy,
                                     bias=MAGIC, scale=qsc)
            else:
                nc.vector.tensor_scalar(v1[:], st[:], qsc, MAGIC,
                                        mybir.AluOpType.mult,
                                        mybir.AluOpType.add)
            qq = qp.tile([128, KP], BF16, tag="q")
            nc.vector.memset(qq[:, D_IN:], 0.0)
            nc.vector.tensor_scalar_sub(qq[:, :D_IN], v1[:], MAGIC)
            for hh in range(2):
                ps = ptr.tile([128, KP // 2], BF16, tag=f"tr{hh}")
                for k8 in range(NK // 2):
                    k = hh * (NK // 2) + k8
                    nc.tensor.transpose(
                        ps[:, k8 * 128:(k8 + 1) * 128],
                        qq[:, k * 128:(k + 1) * 128], idb[:])
                dst = qts[:, tq * KP + hh * (KP // 2):
                          tq * KP + (hh + 1) * (KP // 2)]
                if hh == 0:
                    nc.scalar.activation(
                        dst, ps[:], mybir.ActivationFunctionType.Copy,
                        bias=0.0, scale=1.0)
                else:
                    nc.vector.tensor_copy(dst, ps[:])
        cols = slice(qbase * 128, (qbase + qn) * 128)
        qbase += qn
        for k in range(NK):
            nc.tensor.matmul(ps_a1[:, cols], w1c[k][:],
                             qtsv[:, :, k, :],
                             start=(k == 0), stop=(k == NK - 1))
        # relu+bias then per-quarter feature stats
        nc.scalar.activation(r[:, cols], ps_a1[:, cols],
                             mybir.ActivationFunctionType.Relu,
                             bias=b1_sb[:], scale=1.0)
        nc.vector.reduce_sum(st_s[:, q:q + 1], r[:, cols],
                             axis=mybir.AxisListType.X)
        nc.scalar.activation(sqh[:, cols], r[:, cols],
                             mybir.ActivationFunctionType.Square,
                             accum_out=st_q[:, q:q + 1])
        nc.vector.reduce_max(st_m[:, q:q + 1], r[:, cols],
                             axis=mybir.AxisListType.X)
        if q == 0:
            # preload the SQRT activation table off the critical path
            dum = sp.tile([1, 1], F32, tag="dum")
            nc.scalar.activation(dum[:], one1[:],
                                 mybir.ActivationFunctionType.Sqrt)

    # ---------- final local stats + one AllGather ----------
    stat3 = sp.tile([HID, 3], F32, tag="stat3")
    nc.vector.reduce_sum(stat3[:, 0:1], st_s[:], axis=mybir.AxisListType.X)
    nc.vector.reduce_sum(stat3[:, 1:2], st_q[:], axis=mybir.AxisListType.X)
    nc.vector.reduce_max(stat3[:, 2:3], st_m[:], axis=mybir.AxisListType.X)

    din = dcc.tile([HID, 3], F32, tag="di_ag")
    dout = dcc.tile([NCORES * HID, 3], F32, tag="do_ag")
    nc.sync.dma_start(din[:], stat3[:])
    nc.gpsimd.collective_compute(
        "AllGather", mybir.AluOpType.bypass, replica_groups=rg,
        ins=[din.opt()], outs=[dout.opt()])
    g8 = sp.tile([HID, NCORES * 3], F32, tag="g8")
    nc.sync.dma_start(
        g8[:].rearrange("p (c s) -> p c s", c=NCORES),
        dout[:].rearrange("(c p) s -> p c s", p=HID))
    # strided reduces across the 8 core-chunks
    g8v = g8[:].rearrange("p (c s) -> p s c", c=NCORES)
    sums = sp.tile([HID, 2], F32, tag="sums")
    nc.vector.reduce_sum(sums[:], g8v[:, 0:2, :], axis=mybir.AxisListType.X)
    rmaxf = sp.tile([HID, 1], F32, tag="rmaxf")
    nc.vector.reduce_max(rmaxf[:], g8v[:, 2:3, :], axis=mybir.AxisListType.X)
    # collapse per-feature max -> global maxr
    ps_m = pss.tile([1, HID], F32, tag="psm")
    nc.tensor.transpose(ps_m[:], rmaxf[:], id_sb[:HID, :HID])
    maxr = sp.tile([1, 1], F32, tag="maxr")
    nc.vector.reduce_max(maxr[:], ps_m[:], axis=mybir.AxisListType.X)

    # ---------- BN affine folded into linear2 coefficients ----------
    rrm = sp.tile([1, 1], F32, tag="rrm")
    nc.vector.reciprocal(rrm[:], maxr[:])
    qsc2 = bcast(rrm, HID, 127.0, "qsc2")         # [HID,1] = 127/maxr
    # quantize r per 512-chunk (Scalar; parallel to Vector fold chain below)
    for n in range(4):
        cn = slice(n * 512, (n + 1) * 512)
        nc.scalar.activation(r[:, cn], r[:, cn],
                             mybir.ActivationFunctionType.Copy,
                             bias=MAGIC, scale=qsc2[:])

    m12 = sp.tile([HID, 2], F32, tag="m12")
    nc.vector.tensor_scalar_mul(m12[:], sums[:], 1.0 / BATCH)
    mu2 = sp.tile([HID, 1], F32, tag="mu2")
    nc.scalar.square(mu2[:], m12[:, 0:1])
    varr = sp.tile([HID, 1], F32, tag="varr")
    nc.vector.tensor_tensor(varr[:], m12[:, 1:2], mu2[:],
                            mybir.AluOpType.subtract)
    # + (maxr/(127*sqrt(12)))^2 : rounding variance of q2 in r-units
    rv = sp.tile([1, 1], F32, tag="rv")
    nc.scalar.activation(rv[:], maxr[:], mybir.ActivationFunctionType.Square,
                         scale=1.0 / (127.0 * np.sqrt(12.0)))
    rvb = bcast(rv, HID, 1.0, "rvb")
    nc.vector.tensor_tensor(varr[:], varr[:], rvb[:], mybir.AluOpType.add)
    # sd = sqrt(varr*S1^2 + eps)
    epst = sp.tile([HID, 1], F32, tag="epst")
    nc.vector.memset(epst[:], BN_EPS)
    sd = sp.tile([HID, 1], F32, tag="sd")
    nc.scalar.activation(sd[:], varr[:], mybir.ActivationFunctionType.Sqrt,
                         bias=epst[:], scale=S1 * S1)
    isd = sp.tile([HID, 1], F32, tag="isd")
    nc.vector.reciprocal(isd[:], sd[:])
    abn = sp.tile([HID, 1], F32, tag="abn")
    nc.vector.tensor_tensor(abn[:], gam_sb[:], isd[:], mybir.AluOpType.mult)
    mu = sp.tile([HID, 1], F32, tag="mu")
    nc.vector.tensor_scalar_mul(mu[:], m12[:, 0:1], S1)
    amu = sp.tile([HID, 1], F32, tag="amu")
    nc.vector.tensor_tensor(amu[:], abn[:], mu[:], mybir.AluOpType.mult)
    cbn = sp.tile([HID, 1], F32, tag="cbn")
    nc.vector.tensor_tensor(cbn[:], bet_sb[:], amu[:],
                            mybir.AluOpType.subtract)
    # abns = abn*s2*w2s ;  y = (q2 ints) * abns  (f32, exact products)
    s2t = sp.tile([1, 1], F32, tag="s2t")
    nc.vector.tensor_scalar_mul(s2t[:], maxr[:], S1 / 127.0)
    s2b = bcast(s2t, HID, 1.0, "s2b")
    abns = sp.tile([HID, 1], F32, tag="abns")
    nc.vector.tensor_scalar(abns[:], abn[:], s2b[:], w2s,
                            mybir.AluOpType.mult, mybir.AluOpType.mult)
    y = hp.tile([HID, SHARD], F32, tag="sqh")
    # zbias[1,2] = w2s*(cbn @ w2int) + b2i*(w2s*s2)
    ps_zb = pss.tile([1, OUT], F32, tag="psm")
    nc.tensor.matmul(ps_zb[:], cbn[:], w2f[:], start=True, stop=True)
    zb1 = sp.tile([1, OUT], F32, tag="zb1")
    nc.vector.tensor_scalar_mul(zb1[:], ps_zb[:], w2s)
    rs2 = sp.tile([1, 1], F32, tag="rs2")
    nc.vector.reciprocal(rs2[:], s2t[:])
    b2sc = sp.tile([1, 1], F32, tag="b2sc")
    nc.vector.tensor_scalar_mul(b2sc[:], rs2[:], 1.0 / w2s)  # 1/(w2s*s2)
    t3 = sp.tile([1, OUT], F32, tag="t3")
    nc.scalar.activation(t3[:], b2_sb[:], mybir.ActivationFunctionType.Copy,
                         bias=MAGIC, scale=b2sc[:])
    b2i = sp.tile([1, OUT], F32, tag="b2i")
    nc.vector.tensor_scalar(b2i[:], t3[:], MAGIC, 1.0,
                            mybir.AluOpType.subtract, mybir.AluOpType.min)
    nc.vector.tensor_scalar_max(b2i[:], b2i[:], -2.0)
    b2is = sp.tile([1, OUT], F32, tag="b2is")
    nc.vector.tensor_scalar(b2is[:], b2i[:], s2t[:], w2s,
                            mybir.AluOpType.mult, mybir.AluOpType.mult)
    zbias = sp.tile([1, OUT], F32, tag="zbias")
    nc.vector.tensor_tensor(zbias[:], zb1[:], b2is[:], mybir.AluOpType.add)
    ps_zbt = pss.tile([OUT, 1], F32, tag="psm")
    nc.tensor.transpose(ps_zbt[:], zbias[:], one1[:])
    zb2 = sp.tile([OUT, 1], F32, tag="zb2")
    nc.vector.tensor_copy(zb2[:], ps_zbt[:])

    # ---------- y + GEMM2 (f32) + relu + max, pipelined per 512-chunk ------
    ps_z = psb.tile([OUT, SHARD], F32, tag="big")
    zr = zp.tile([OUT, SHARD], F32, tag="z")
    zmx = sp.tile([OUT, 4], F32, tag="zmx")
    for n in range(4):
        cn = slice(n * 512, (n + 1) * 512)
        nc.vector.tensor_scalar(y[:, cn], r[:, cn], MAGIC, abns[:],
                                mybir.AluOpType.subtract,
                                mybir.AluOpType.mult)
        nc.tensor.matmul(ps_z[:, cn], w2f[:], y[:, cn],
                         start=True, stop=True)
        if n < 2:
            nc.scalar.activation(zr[:, cn], ps_z[:, cn],
                                 mybir.ActivationFunctionType.Relu,
                                 bias=zb2[:], scale=1.0)
        else:
            nc.vector.tensor_scalar(zr[:, cn], ps_z[:, cn], zb2[:], 0.0,
                                    mybir.AluOpType.add,
                                    mybir.AluOpType.max)
        nc.vector.reduce_max(zmx[:, n:n + 1], zr[:, cn],
                             axis=mybir.AxisListType.X)
    zm1 = sp.tile([OUT, 1], F32, tag="zm1")
    nc.vector.reduce_max(zm1[:], zmx[:], axis=mybir.AxisListType.X)
    ps_c = pss.tile([1, OUT], F32, tag="psm")
    nc.tensor.transpose(ps_c[:], zm1[:], id_sb[:OUT, :OUT])
    lmz = sp.tile([1, 1], F32, tag="lmz")
    nc.vector.reduce_max(lmz[:], ps_c[:], axis=mybir.AxisListType.X)

    din2 = dcc.tile([1, 1], F32, tag="di_ag2")
    dout2 = dcc.tile([NCORES, 1], F32, tag="do_ag2")
    nc.sync.dma_start(din2[:], lmz[:])
    nc.gpsimd.collective_compute(
        "AllGather", mybir.AluOpType.bypass, replica_groups=rg,
        ins=[din2.opt()], outs=[dout2.opt()])
    g4 = sp.tile([NCORES, 1], F32, tag="g4")
    nc.sync.dma_start(g4[:], dout2[:])
    ps_c2 = pss.tile([1, NCORES], F32, tag="psm")
    nc.tensor.transpose(ps_c2[:], g4[:], id_sb[:NCORES, :NCORES])
    gmz = sp.tile([1, 1], F32, tag="gmz")
    nc.vector.reduce_max(gmz[:], ps_c2[:], axis=mybir.AxisListType.X)

    # ---------- final quant + store ----------
    HS = SHARD // 2
    rmz = sp.tile([1, 1], F32, tag="rmz")
    nc.vector.reciprocal(rmz[:], gmz[:])
    qsc3 = bcast(rmz, OUT, 127.0, "qsc3")          # [2,1] 127/maxz
    s3b = bcast(gmz, OUT, 1.0 / 127.0, "s3b")      # [2,1] maxz/127
    t5 = zp.tile([OUT, SHARD], F32, tag="z")
    nc.scalar.activation(t5[:, :HS], zr[:, :HS],
                         mybir.ActivationFunctionType.Copy,
                         bias=MAGIC, scale=qsc3[:])
    nc.vector.tensor_scalar(t5[:, HS:], zr[:, HS:], qsc3[:], MAGIC,
                            mybir.AluOpType.mult, mybir.AluOpType.add)
    osb = zp.tile([OUT, SHARD], F32, tag="z")
    nc.vector.tensor_scalar(osb[:, HS:], t5[:, HS:], MAGIC, s3b[:],
                            mybir.AluOpType.subtract, mybir.AluOpType.mult)
    nc.scalar.activation(osb[:, :HS], t5[:, :HS],
                         mybir.ActivationFunctionType.Copy,
                         bias=-MAGIC, scale=1.0)
    nc.vector.tensor_scalar_mul(osb[:, :HS], osb[:, :HS], s3b[:])
    nc.scalar.dma_start(out[:, HS:], osb[:, HS:])
    nc.sync.dma_start(out[:, :HS], osb[:, :HS])


def _prep(sig, W1, b1, W2, b2, gamma, beta):
    sig = np.ascontiguousarray(np.asarray(sig, dtype=np.float32))
    W1 = np.asarray(W1, dtype=np.float32)
    W2 = np.asarray(W2, dtype=np.float32)
    b1 = np.asarray(b1, dtype=np.float32)
    gmax = float(np.max(np.abs(sig)))
    w1s = float(np.max(np.abs(W1)))
    w2s = float(np.max(np.abs(W2)))
    w1i = np.clip(np.round(W1 / w1s), -2, 1).astype(np.float32)
    w2i = np.clip(np.round(W2 / w2s), -2, 1).astype(np.float32)
    b1q = np.clip(np.round(b1 * (15.0 / (w1s * gmax))), -2, 1)
    w1t = np.zeros((KP, HID), dtype=ml_dtypes.bfloat16)
    w1t[:D_IN, :] = w1i.T.astype(ml_dtypes.bfloat16)
    w2t = np.ascontiguousarray(w2i.T).astype(np.float32)
    com = {
        "w1t": w1t,
        "w2t": w2t,
        "b1i": b1q.astype(np.float32).reshape(HID, 1),
        "b2": np.ascontiguousarray(np.asarray(b2, np.float32).reshape(1, OUT)),
        "gamma": np.asarray(gamma, np.float32).reshape(HID, 1),
        "bet": np.asarray(beta, np.float32).reshape(HID, 1),
        "ident": np.eye(128, dtype=np.float32),
    }
    in_maps = []
    for c in range(NCORES):
        m = dict(com)
        m["sig"] = np.ascontiguousarray(sig[c * SHARD:(c + 1) * SHARD])
        in_maps.append(m)
    return w1s, w2s, gmax, in_maps


def kernel(sig, W1, b1, W2, b2, gamma, beta):
    w1s, w2s, gmax, in_maps = _prep(sig, W1, b1, W2, b2, gamma, beta)
    key = (round(w1s, 9), round(w2s, 9), round(gmax, 9))
    if key not in _CACHE:
        _CACHE[key] = _build(w1s, w2s, gmax)
    nc = _CACHE[key]
    trace = os.environ.get("BASS_TRACE") == "1"
    try:
        res = bass_utils.run_bass_kernel_spmd(
            nc, in_maps, core_ids=list(range(NCORES)), trace=trace)
    except ModuleNotFoundError:
        res = bass_utils.run_bass_kernel_spmd(
            nc, in_maps, core_ids=list(range(NCORES)), trace=False)
    kernel.last_results = res
    return np.concatenate([r["out"].T for r in res.results], axis=0)


# revision 25
# speedup vs baseline: 1.1247x; 1.1247x over previous
"""HAWQ tiny classifier on 8 TRN2 cores — pure data parallel, v3.

Per core: batch shard [2048, 2000].
Host precomputes: gmax=max|sig|, w1s/w2s weight scales, int weights (bf16),
b1 integer quantization. Device pipeline (tile-granular, no DRAM bounce):
  q   = round(sig * 15/gmax)          bf16 in SBUF
  qT  = PE-transpose of q (128x128 blocks via identity matmuls -> PSUM),
        copied PSUM->SBUF per half-tile (Scalar), quarter-batched GEMM1
  r   = relu(a1 + b1int); per-quarter feature stats [sum, sumsq, max]
  ONE AllGather of [100,3] stats (warmup collective absorbs launch skew)
  BN folded with approx stats (rounding-variance corrected: var += s2^2/12)
  y   = round(r*127/maxr) * (abn*s2*w2s);  z = w2f.T @ y + zbias  (f32 PE,
        8 batch-chunks written at psum partition offsets -> [16,256] layout)
  zr = relu(z); AllGather #2 of maxz; out = round(zr*127/maxz)*maxz/127
Output written [16, 256] per core; host reshapes/concats.
Rounding uses the f32 magic-number trick (+1.5*2^23 = round-to-nearest-even).
"""

import os
import sys

for p in ("/opt/trn_rl_repo", "/opt/trn_rl_repo/concourse"):
    if p not in sys.path:
        sys.path.insert(0, p)

import numpy as np
import ml_dtypes

import concourse.bass as bass
import concourse.bacc as bacc
import concourse.tile as tile
import concourse.mybir as mybir
from concourse import bass_utils
from concourse._compat import with_exitstack

F32 = mybir.dt.float32
BF16 = mybir.dt.bfloat16

BATCH, D_IN, HID, OUT = 16384, 2000, 100, 2
NCORES = 8
SHARD = BATCH // NCORES          # 2048 rows per core
NT = SHARD // 128                # 16 batch tiles per core
NQ = 5                           # phase-B chunks
QT = NT // NQ                    # 4 tiles per quarter
QROWS = SHARD // NQ              # 512 rows per quarter
KP = 2048                        # padded contraction dim (2000 -> 16*128)
NK = KP // 128                   # 16 k-chunks
MAGIC = 12582912.0               # 1.5 * 2**23
BN_EPS = 1e-5
NZ = 4                           # batch chunks for layer-2 output
ZC = SHARD // NZ                 # 512 cols per z chunk

_CACHE = {}


def _build(w1s: float, w2s: float, gmax: float):
    nc = bacc.Bacc(
        "TRN2",
        target_bir_lowering=False,
        debug=False,
        enable_asserts=False,
        num_devices=NCORES,
    )

    sig = nc.dram_tensor("sig", [SHARD, D_IN], F32, kind="ExternalInput")
    w1t = nc.dram_tensor("w1t", [KP, HID], BF16, kind="ExternalInput")
    w2t = nc.dram_tensor("w2t", [HID, OUT], F32, kind="ExternalInput")
    b1i = nc.dram_tensor("b1i", [HID, 1], F32, kind="ExternalInput")
    b2 = nc.dram_tensor("b2", [1, OUT], F32, kind="ExternalInput")
    gam = nc.dram_tensor("gamma", [HID, 1], F32, kind="ExternalInput")
    bet = nc.dram_tensor("bet", [HID, 1], F32, kind="ExternalInput")
    ident = nc.dram_tensor("ident", [128, 128], F32, kind="ExternalInput")
    out = nc.dram_tensor("out", [OUT, SHARD], F32, kind="ExternalOutput")
    rg = [list(range(NCORES))]

    with tile.TileContext(nc) as tc:
        _kern(tc, nc, sig, w1t, w2t, b1i, b2, gam, bet, ident, out,
              rg, w1s, w2s, gmax)
    nc.compile()
    return nc


@with_exitstack
def _kern(ctx, tc, nc, sig, w1t, w2t, b1i, b2, gam, bet, ident, out, rg,
          w1s, w2s, gmax):
    S1 = w1s * gmax / 15.0           # scale of r (real = r*S1)

    sigp = ctx.enter_context(tc.tile_pool(name="sigp", bufs=3))
    tmpp = ctx.enter_context(tc.tile_pool(name="tmpp", bufs=2))
    qp = ctx.enter_context(tc.tile_pool(name="qp", bufs=3))
    qtsp = ctx.enter_context(tc.tile_pool(name="qtsp", bufs=2))
    wp = ctx.enter_context(tc.tile_pool(name="wp", bufs=1))
    hp = ctx.enter_context(tc.tile_pool(name="hp", bufs=1))      # [HID,2048]
    sp = ctx.enter_context(tc.tile_pool(name="sp", bufs=1))      # small stats
    zp = ctx.enter_context(tc.tile_pool(name="zp", bufs=2))      # [16,256]
    psb = ctx.enter_context(tc.tile_pool(name="psb", bufs=1, space="PSUM"))
    ptr = ctx.enter_context(tc.tile_pool(name="ptr", bufs=1, space="PSUM"))
    pss = ctx.enter_context(tc.tile_pool(name="pss", bufs=2, space="PSUM"))
    dcc = ctx.enter_context(tc.tile_pool(name="dcc", bufs=1, space="DRAM"))

    # ---- prologue: small loads (SP queue) ----
    id_sb = sp.tile([128, 128], F32, tag="ident")
    nc.sync.dma_start(id_sb[:], ident[:, :])
    idb = sp.tile([128, 128], BF16, tag="idb")
    nc.vector.tensor_copy(idb[:], id_sb[:])
    one1 = sp.tile([1, 1], F32, tag="one1")
    nc.vector.memset(one1[:], 1.0)
    b1_sb = sp.tile([HID, 1], F32, tag="b1i")
    nc.sync.dma_start(b1_sb[:], b1i[:, :])
    b2_sb = sp.tile([1, OUT], F32, tag="b2")
    nc.sync.dma_start(b2_sb[:], b2[:, :])
    gam_sb = sp.tile([HID, 1], F32, tag="gam")
    nc.sync.dma_start(gam_sb[:], gam[:, :])
    bet_sb = sp.tile([HID, 1], F32, tag="bet")
    nc.sync.dma_start(bet_sb[:], bet[:, :])
    w2f = sp.tile([HID, OUT], F32, tag="w2f")
    nc.sync.dma_start(w2f[:], w2t[:, :])
    w1c = []
    for k in range(NK):
        wt = wp.tile([128, HID], BF16, tag=f"w1_{k}")
        nc.sync.dma_start(wt[:], w1t[k * 128:(k + 1) * 128, :])
        w1c.append(wt)
    # warmup collective: absorbs cross-core launch skew off the critical path
    wdin = dcc.tile([1, 1], F32, tag="wu_i")
    wdout = dcc.tile([NCORES, 1], F32, tag="wu_o")
    nc.sync.dma_start(wdin[:], one1[:])
    nc.gpsimd.collective_compute(
        "AllGather", mybir.AluOpType.bypass, replica_groups=rg,
        ins=[wdin.opt()], outs=[wdout.opt()])

    # ---------- helpers ----------
    def bcast(scal, n, val, tag):
        """[n,1] f32 = val * scal (scal is [1,1])."""
        r = sp.tile([n, 1], F32, tag=tag)
        nc.gpsimd.partition_broadcast(r[:], scal[:])
        if val != 1.0:
            nc.vector.tensor_scalar_mul(r[:], r[:], float(val))
        return r

    # ---------- phase B+C: load, quantize, PE-transpose, GEMM1 ----------
    qsc = 15.0 / gmax
    ps_a1 = psb.tile([HID, SHARD], F32, tag="big")
    r = hp.tile([HID, SHARD], F32, tag="h")
    sqh = hp.tile([HID, SHARD], F32, tag="sqh")
    st_s = sp.tile([HID, NQ], F32, tag="st_s")
    st_q = sp.tile([HID, NQ], F32, tag="st_q")
    st_m = sp.tile([HID, NQ], F32, tag="st_m")

    QSIZES = (4, 4, 4, 3, 1)
    qbase = 0
    for q in range(NQ):
        qn = QSIZES[q]
        qts = qtsp.tile([128, NK * qn * 128], BF16, tag="qts")
        # layout [p, (t k c)]: tile-major -> contiguous PSUM->SBUF copies
        qtsv = qts[:].rearrange("p (t k c) -> p t k c", t=qn, k=NK)
        for tq in range(qn):
            t = qbase + tq
            st = sigp.tile([128, D_IN], F32, tag="sig")
            eng = nc.scalar if t % 2 == 0 else nc.sync
            eng.dma_start(st[:], sig[t * 128:(t + 1) * 128, :])
            v1 = tmpp.tile([128, D_IN], F32, tag="v1")
            if t % 2 == 0:
                nc.scalar.activation(v1[:], st[:],
                                     mybir.ActivationFunctionType.Copy,
                                     bias=MAGIC, scale=qsc)
            else:
                nc.vector.tensor_scalar(v1[:], st[:], qsc, MAGIC,
                                        mybir.AluOpType.mult,
                                        mybir.AluOpType.add)
            qq = qp.tile([128, KP], BF16, tag="q")
            nc.vector.memset(qq[:, D_IN:], 0.0)
            nc.vector.tensor_scalar_sub(qq[:, :D_IN], v1[:], MAGIC)
            for hh in range(2):
                ps = ptr.tile([128, KP // 2], BF16, tag=f"tr{hh}")
                for k8 in range(NK // 2):
                    k = hh * (NK // 2) + k8
                    nc.tensor.transpose(
                        ps[:, k8 * 128:(k8 + 1) * 128],
                        qq[:, k * 128:(k + 1) * 128], idb[:])
                dst = qts[:, tq * KP + hh * (KP // 2):
                          tq * KP + (hh + 1) * (KP // 2)]
                if hh == 0:
                    nc.scalar.activation(
                        dst, ps[:], mybir.ActivationFunctionType.Copy,
                        bias=0.0, scale=1.0)
                else:
                    nc.vector.tensor_copy(dst, ps[:])
        cols = slice(qbase * 128, (qbase + qn) * 128)
        qbase += qn
        for k in range(NK):
            nc.tensor.matmul(ps_a1[:, cols], w1c[k][:],
                             qtsv[:, :, k, :],
                             start=(k == 0), stop=(k == NK - 1))
        # relu+bias then per-quarter feature stats
        nc.scalar.activation(r[:, cols], ps_a1[:, cols],
                             mybir.ActivationFunctionType.Relu,
                             bias=b1_sb[:], scale=1.0)
        nc.vector.reduce_sum(st_s[:, q:q + 1], r[:, cols],
                             axis=mybir.AxisListType.X)
        nc.scalar.activation(sqh[:, cols], r[:, cols],
                             mybir.ActivationFunctionType.Square,
                             accum_out=st_q[:, q:q + 1])
        nc.vector.reduce_max(st_m[:, q:q + 1], r[:, cols],
                             axis=mybir.AxisListType.X)
        if q == 0:
            # preload the SQRT activation table off the critical path
            dum = sp.tile([1, 1], F32, tag="dum")
            nc.scalar.activation(dum[:], one1[:],
                                 mybir.ActivationFunctionType.Sqrt)

    # ---------- final local stats + one AllGather ----------
    stat3 = sp.tile([HID, 3], F32, tag="stat3")
    nc.vector.reduce_sum(stat3[:, 0:1], st_s[:], axis=mybir.AxisListType.X)
    nc.vector.reduce_sum(stat3[:, 1:2], st_q[:], axis=mybir.AxisListType.X)
    nc.vector.reduce_max(stat3[:, 2:3], st_m[:], axis=mybir.AxisListType.X)

    din = dcc.tile([HID, 3], F32, tag="di_ag")
    dout = dcc.tile([NCORES * HID, 3], F32, tag="do_ag")
    nc.gpsimd.dma_start(din[:], stat3[:])
    nc.gpsimd.collective_compute(
        "AllGather", mybir.AluOpType.bypass, replica_groups=rg,
        ins=[din.opt()], outs=[dout.opt()])
    g8 = sp.tile([HID, NCORES * 3], F32, tag="g8")
    nc.sync.dma_start(
        g8[:].rearrange("p (c s) -> p c s", c=NCORES),
        dout[:].rearrange("(c p) s -> p c s", p=HID))
    # strided reduces across the 8 core-chunks
    g8v = g8[:].rearrange("p (c s) -> p s c", c=NCORES)
    sums = sp.tile([HID, 2], F32, tag="sums")
    nc.vector.reduce_sum(sums[:], g8v[:, 0:2, :], axis=mybir.AxisListType.X)
    rmaxf = sp.tile([HID, 1], F32, tag="rmaxf")
    nc.vector.reduce_max(rmaxf[:], g8v[:, 2:3, :], axis=mybir.AxisListType.X)
    # collapse per-feature max -> global maxr
    ps_m = pss.tile([1, HID], F32, tag="psm")
    nc.tensor.transpose(ps_m[:], rmaxf[:], id_sb[:HID, :HID])
    maxr = sp.tile([1, 1], F32, tag="maxr")
    nc.vector.reduce_max(maxr[:], ps_m[:], axis=mybir.AxisListType.X)

    # ---------- BN affine folded into linear2 coefficients ----------
    rrm = sp.tile([1, 1], F32, tag="rrm")
    nc.vector.reciprocal(rrm[:], maxr[:])
    qsc2 = bcast(rrm, HID, 127.0, "qsc2")         # [HID,1] = 127/maxr
    # quantize r per 1024-chunk (Scalar; parallel to Vector fold chain below)
    for n in range(2):
        cn = slice(n * 1024, (n + 1) * 1024)
        nc.scalar.activation(r[:, cn], r[:, cn],
                             mybir.ActivationFunctionType.Copy,
                             bias=MAGIC, scale=qsc2[:])

    m12 = sp.tile([HID, 2], F32, tag="m12")
    nc.vector.tensor_scalar_mul(m12[:], sums[:], 1.0 / BATCH)
    mu2 = sp.tile([HID, 1], F32, tag="mu2")
    nc.scalar.square(mu2[:], m12[:, 0:1])
    varr = sp.tile([HID, 1], F32, tag="varr")
    nc.vector.tensor_tensor(varr[:], m12[:, 1:2], mu2[:],
                            mybir.AluOpType.subtract)
    # rounding variance of q2 (in r-units, scaled by S1^2) + eps, as the
    # sqrt bias: sd = sqrt(varr*S1^2 + (maxr*S1/(127*sqrt(12)))^2 + eps)
    rv = sp.tile([1, 1], F32, tag="rv")
    nc.scalar.activation(rv[:], maxr[:], mybir.ActivationFunctionType.Square,
                         scale=S1 / (127.0 * np.sqrt(12.0)))
    nc.vector.tensor_scalar_add(rv[:], rv[:], BN_EPS)
    rvb = bcast(rv, HID, 1.0, "rvb")
    sd = sp.tile([HID, 1], F32, tag="sd")
    nc.scalar.activation(sd[:], varr[:], mybir.ActivationFunctionType.Sqrt,
                         bias=rvb[:], scale=S1 * S1)
    isd = sp.tile([HID, 1], F32, tag="isd")
    nc.vector.reciprocal(isd[:], sd[:])
    abn = sp.tile([HID, 1], F32, tag="abn")
    nc.vector.tensor_tensor(abn[:], gam_sb[:], isd[:], mybir.AluOpType.mult)
    mu = sp.tile([HID, 1], F32, tag="mu")
    nc.vector.tensor_scalar_mul(mu[:], m12[:, 0:1], S1)
    amu = sp.tile([HID, 1], F32, tag="amu")
    nc.vector.tensor_tensor(amu[:], abn[:], mu[:], mybir.AluOpType.mult)
    cbn = sp.tile([HID, 1], F32, tag="cbn")
    nc.vector.tensor_tensor(cbn[:], bet_sb[:], amu[:],
                            mybir.AluOpType.subtract)
    # abns = abn*s2*w2s ;  y = (q2 ints) * abns  (f32, exact products)
    s2t = sp.tile([1, 1], F32, tag="s2t")
    nc.vector.tensor_scalar_mul(s2t[:], maxr[:], S1 / 127.0)
    s2b = bcast(s2t, HID, 1.0, "s2b")
    abns = sp.tile([HID, 1], F32, tag="abns")
    nc.vector.tensor_scalar(abns[:], abn[:], s2b[:], w2s,
                            mybir.AluOpType.mult, mybir.AluOpType.mult)
    y = hp.tile([HID, SHARD], F32, tag="sqh")
    # zbias[1,2] = w2s*(cbn @ w2int) + b2i*(w2s*s2)
    ps_zb = pss.tile([1, OUT], F32, tag="psm")
    nc.tensor.matmul(ps_zb[:], cbn[:], w2f[:], start=True, stop=True)
    zb1 = sp.tile([1, OUT], F32, tag="zb1")
    nc.vector.tensor_scalar_mul(zb1[:], ps_zb[:], w2s)
    rs2 = sp.tile([1, 1], F32, tag="rs2")
    nc.vector.reciprocal(rs2[:], s2t[:])
    b2sc = sp.tile([1, 1], F32, tag="b2sc")
    nc.vector.tensor_scalar_mul(b2sc[:], rs2[:], 1.0 / w2s)  # 1/(w2s*s2)
    t3 = sp.tile([1, OUT], F32, tag="t3")
    nc.scalar.activation(t3[:], b2_sb[:], mybir.ActivationFunctionType.Copy,
                         bias=MAGIC, scale=b2sc[:])
    b2i = sp.tile([1, OUT], F32, tag="b2i")
    nc.vector.tensor_scalar(b2i[:], t3[:], MAGIC, 1.0,
                            mybir.AluOpType.subtract, mybir.AluOpType.min)
    nc.vector.tensor_scalar_max(b2i[:], b2i[:], -2.0)
    b2is = sp.tile([1, OUT], F32, tag="b2is")
    nc.vector.tensor_scalar(b2is[:], b2i[:], s2t[:], w2s,
                            mybir.AluOpType.mult, mybir.AluOpType.mult)
    zbias = sp.tile([1, OUT], F32, tag="zbias")
    nc.vector.tensor_tensor(zbias[:], zb1[:], b2is[:], mybir.AluOpType.add)
    ps_zbt = pss.tile([OUT, 1], F32, tag="psm")
    nc.tensor.transpose(ps_zbt[:], zbias[:], one1[:])
    zb2 = sp.tile([OUT, 1], F32, tag="zb2")
    nc.vector.tensor_copy(zb2[:], ps_zbt[:])

    # ---------- y + GEMM2 (f32) + relu + max, pipelined per 512-chunk ------
    ps_z = psb.tile([OUT, SHARD], F32, tag="big")
    zr = zp.tile([OUT, SHARD], F32, tag="z")
    zmx = sp.tile([OUT, 2], F32, tag="zmx")
    for n in range(2):
        cn = slice(n * 1024, (n + 1) * 1024)
        nc.vector.tensor_scalar(y[:, cn], r[:, cn], MAGIC, abns[:],
                                mybir.AluOpType.subtract,
                                mybir.AluOpType.mult)
        for m in range(2):
            cm = slice(n * 1024 + m * 512, n * 1024 + (m + 1) * 512)
            nc.tensor.matmul(ps_z[:, cm], w2f[:], y[:, cm],
                             start=True, stop=True)
        if n == 0:
            nc.scalar.activation(zr[:, cn], ps_z[:, cn],
                                 mybir.ActivationFunctionType.Relu,
                                 bias=zb2[:], scale=1.0)
        else:
            nc.vector.tensor_scalar(zr[:, cn], ps_z[:, cn], zb2[:], 0.0,
                                    mybir.AluOpType.add,
                                    mybir.AluOpType.max)
        nc.vector.reduce_max(zmx[:, n:n + 1], zr[:, cn],
                             axis=mybir.AxisListType.X)
    zm2 = sp.tile([OUT, 1], F32, tag="zm2")
    nc.vector.reduce_max(zm2[:], zmx[:], axis=mybir.AxisListType.X)

    din2 = dcc.tile([OUT, 1], F32, tag="di_ag2")
    dout2 = dcc.tile([NCORES * OUT, 1], F32, tag="do_ag2")
    nc.gpsimd.dma_start(din2[:], zm2[:])
    nc.gpsimd.collective_compute(
        "AllGather", mybir.AluOpType.bypass, replica_groups=rg,
        ins=[din2.opt()], outs=[dout2.opt()])
    g4r = sp.tile([1, NCORES * OUT], F32, tag="g4r")
    nc.sync.dma_start(g4r[:], dout2[:].rearrange("a b -> b a"))
    gmz = sp.tile([1, 1], F32, tag="gmz")
    nc.vector.reduce_max(gmz[:], g4r[:], axis=mybir.AxisListType.X)

    # ---------- final quant + store ----------
    HS = SHARD // 2
    rmz = sp.tile([1, 1], F32, tag="rmz")
    nc.vector.reciprocal(rmz[:], gmz[:])
    qsc3 = bcast(rmz, OUT, 127.0, "qsc3")          # [2,1] 127/maxz
    s3b = bcast(gmz, OUT, 1.0 / 127.0, "s3b")      # [2,1] maxz/127
    t5 = zp.tile([OUT, SHARD], F32, tag="z")
    nc.scalar.activation(t5[:, :HS], zr[:, :HS],
                         mybir.ActivationFunctionType.Copy,
                         bias=MAGIC, scale=qsc3[:])
    nc.vector.tensor_scalar(t5[:, HS:], zr[:, HS:], qsc3[:], MAGIC,
                            mybir.AluOpType.mult, mybir.AluOpType.add)
    osb = zp.tile([OUT, SHARD], F32, tag="z")
    nc.vector.tensor_scalar(osb[:, HS:], t5[:, HS:], MAGIC, s3b[:],
                            mybir.AluOpType.subtract, mybir.AluOpType.mult)
    nc.scalar.activation(osb[:, :HS], t5[:, :HS],
                         mybir.ActivationFunctionType.Copy,
                         bias=-MAGIC, scale=1.0)
    nc.vector.tensor_scalar_mul(osb[:, :HS], osb[:, :HS], s3b[:])
    nc.scalar.dma_start(out[:, HS:], osb[:, HS:])
    nc.sync.dma_start(out[:, :HS], osb[:, :HS])


def _prep(sig, W1, b1, W2, b2, gamma, beta):
    sig = np.ascontiguousarray(np.asarray(sig, dtype=np.float32))
    W1 = np.asarray(W1, dtype=np.float32)
    W2 = np.asarray(W2, dtype=np.float32)
    b1 = np.asarray(b1, dtype=np.float32)
    gmax = float(np.max(np.abs(sig)))
    w1s = float(np.max(np.abs(W1)))
    w2s = float(np.max(np.abs(W2)))
    w1i = np.clip(np.round(W1 / w1s), -2, 1).astype(np.float32)
    w2i = np.clip(np.round(W2 / w2s), -2, 1).astype(np.float32)
    b1q = np.clip(np.round(b1 * (15.0 / (w1s * gmax))), -2, 1)
    w1t = np.zeros((KP, HID), dtype=ml_dtypes.bfloat16)
    w1t[:D_IN, :] = w1i.T.astype(ml_dtypes.bfloat16)
    w2t = np.ascontiguousarray(w2i.T).astype(np.float32)
    com = {
        "w1t": w1t,
        "w2t": w2t,
        "b1i": b1q.astype(np.float32).reshape(HID, 1),
        "b2": np.ascontiguousarray(np.asarray(b2, np.float32).reshape(1, OUT)),
        "gamma": np.asarray(gamma, np.float32).reshape(HID, 1),
        "bet": np.asarray(beta, np.float32).reshape(HID, 1),
        "ident": np.eye(128, dtype=np.float32),
    }
    in_maps = []
    for c in range(NCORES):
        m = dict(com)
        m["sig"] = np.ascontiguousarray(sig[c * SHARD:(c + 1) * SHARD])
        in_maps.append(m)
    return w1s, w2s, gmax, in_maps


def kernel(sig, W1, b1, W2, b2, gamma, beta):
    w1s, w2s, gmax, in_maps = _prep(sig, W1, b1, W2, b2, gamma, beta)
    key = (round(w1s, 9), round(w2s, 9), round(gmax, 9))
    if key not in _CACHE:
        _CACHE[key] = _build(w1s, w2s, gmax)
    nc = _CACHE[key]
    trace = os.environ.get("BASS_TRACE") == "1"
    try:
        res = bass_utils.run_bass_kernel_spmd(
            nc, in_maps, core_ids=list(range(NCORES)), trace=trace)
    except ModuleNotFoundError:
        res = bass_utils.run_bass_kernel_spmd(
            nc, in_maps, core_ids=list(range(NCORES)), trace=False)
    kernel.last_results = res
    return np.concatenate([r["out"].T for r in res.results], axis=0)


# revision 27
# speedup vs baseline: 1.1449x; 1.0180x over previous
"""HAWQ tiny classifier on 8 TRN2 cores — pure data parallel, v3.

Per core: batch shard [2048, 2000].
Host precomputes: gmax=max|sig|, w1s/w2s weight scales, int weights (bf16),
b1 integer quantization. Device pipeline (tile-granular, no DRAM bounce):
  q   = round(sig * 15/gmax)          bf16 in SBUF
  qT  = PE-transpose of q (128x128 blocks via identity matmuls -> PSUM),
        copied PSUM->SBUF per half-tile (Scalar), quarter-batched GEMM1
  r   = relu(a1 + b1int); per-quarter feature stats [sum, sumsq, max]
  ONE AllGather of [100,3] stats (warmup collective absorbs launch skew)
  BN folded with approx stats (rounding-variance corrected: var += s2^2/12)
  y   = round(r*127/maxr) * (abn*s2*w2s);  z = w2f.T @ y + zbias  (f32 PE,
        8 batch-chunks written at psum partition offsets -> [16,256] layout)
  zr = relu(z); AllGather #2 of maxz; out = round(zr*127/maxz)*maxz/127
Output written [16, 256] per core; host reshapes/concats.
Rounding uses the f32 magic-number trick (+1.5*2^23 = round-to-nearest-even).
"""

import os
import sys

for p in ("/opt/trn_rl_repo", "/opt/trn_rl_repo/concourse"):
    if p not in sys.path:
        sys.path.insert(0, p)

import numpy as np
import ml_dtypes

import concourse.bass as bass
import concourse.bacc as bacc
import concourse.tile as tile
import concourse.mybir as mybir
from concourse import bass_utils
from concourse._compat import with_exitstack

F32 = mybir.dt.float32
BF16 = mybir.dt.bfloat16

BATCH, D_IN, HID, OUT = 16384, 2000, 100, 2
NCORES = 8
SHARD = BATCH // NCORES          # 2048 rows per core
NT = SHARD // 128                # 16 batch tiles per core
NQ = 5                           # phase-B chunks
QT = NT // NQ                    # 4 tiles per quarter
QROWS = SHARD // NQ              # 512 rows per quarter
KP = 2048                        # padded contraction dim (2000 -> 16*128)
NK = KP // 128                   # 16 k-chunks
MAGIC = 12582912.0               # 1.5 * 2**23
BN_EPS = 1e-5
NZ = 4                           # batch chunks for layer-2 output
ZC = SHARD // NZ                 # 512 cols per z chunk

_CACHE = {}


def _build(w1s: float, w2s: float, gmax: float):
    nc = bacc.Bacc(
        "TRN2",
        target_bir_lowering=False,
        debug=False,
        enable_asserts=False,
        num_devices=NCORES,
    )

    sig = nc.dram_tensor("sig", [SHARD, D_IN], F32, kind="ExternalInput")
    w1t = nc.dram_tensor("w1t", [KP, HID], BF16, kind="ExternalInput")
    w2t = nc.dram_tensor("w2t", [HID, OUT], F32, kind="ExternalInput")
    b1i = nc.dram_tensor("b1i", [HID, 1], F32, kind="ExternalInput")
    b2 = nc.dram_tensor("b2", [1, OUT], F32, kind="ExternalInput")
    gam = nc.dram_tensor("gamma", [HID, 1], F32, kind="ExternalInput")
    bet = nc.dram_tensor("bet", [HID, 1], F32, kind="ExternalInput")
    ident = nc.dram_tensor("ident", [128, 128], F32, kind="ExternalInput")
    out = nc.dram_tensor("out", [OUT, SHARD], F32, kind="ExternalOutput")
    rg = [list(range(NCORES))]

    with tile.TileContext(nc) as tc:
        _kern(tc, nc, sig, w1t, w2t, b1i, b2, gam, bet, ident, out,
              rg, w1s, w2s, gmax)
    nc.compile()
    return nc


@with_exitstack
def _kern(ctx, tc, nc, sig, w1t, w2t, b1i, b2, gam, bet, ident, out, rg,
          w1s, w2s, gmax):
    S1 = w1s * gmax / 15.0           # scale of r (real = r*S1)

    sigp = ctx.enter_context(tc.tile_pool(name="sigp", bufs=3))
    tmpp = ctx.enter_context(tc.tile_pool(name="tmpp", bufs=2))
    qp = ctx.enter_context(tc.tile_pool(name="qp", bufs=3))
    qtsp = ctx.enter_context(tc.tile_pool(name="qtsp", bufs=2))
    wp = ctx.enter_context(tc.tile_pool(name="wp", bufs=1))
    hp = ctx.enter_context(tc.tile_pool(name="hp", bufs=1))      # [HID,2048]
    sp = ctx.enter_context(tc.tile_pool(name="sp", bufs=1))      # small stats
    zp = ctx.enter_context(tc.tile_pool(name="zp", bufs=2))      # [16,256]
    psb = ctx.enter_context(tc.tile_pool(name="psb", bufs=1, space="PSUM"))
    ptr = ctx.enter_context(tc.tile_pool(name="ptr", bufs=1, space="PSUM"))
    pss = ctx.enter_context(tc.tile_pool(name="pss", bufs=2, space="PSUM"))
    dcc = ctx.enter_context(tc.tile_pool(name="dcc", bufs=1, space="DRAM"))

    # ---- prologue: small loads (SP queue) ----
    id_sb = sp.tile([128, 128], F32, tag="ident")
    nc.sync.dma_start(id_sb[:], ident[:, :])
    idb = sp.tile([128, 128], BF16, tag="idb")
    nc.vector.tensor_copy(idb[:], id_sb[:])
    one1 = sp.tile([1, 1], F32, tag="one1")
    nc.vector.memset(one1[:], 1.0)
    b1_sb = sp.tile([HID, 1], F32, tag="b1i")
    nc.sync.dma_start(b1_sb[:], b1i[:, :])
    b2_sb = sp.tile([1, OUT], F32, tag="b2")
    nc.sync.dma_start(b2_sb[:], b2[:, :])
    gam_sb = sp.tile([HID, 1], F32, tag="gam")
    nc.sync.dma_start(gam_sb[:], gam[:, :])
    bet_sb = sp.tile([HID, 1], F32, tag="bet")
    nc.sync.dma_start(bet_sb[:], bet[:, :])
    w2f = sp.tile([HID, OUT], F32, tag="w2f")
    nc.sync.dma_start(w2f[:], w2t[:, :])
    w1c = []
    for k in range(NK):
        wt = wp.tile([128, HID], BF16, tag=f"w1_{k}")
        w1c.append(wt)
    # warmup collective: absorbs cross-core launch skew off the critical path
    wdin = dcc.tile([1, 1], F32, tag="wu_i")
    wdout = dcc.tile([NCORES, 1], F32, tag="wu_o")
    nc.sync.dma_start(wdin[:], one1[:])
    nc.gpsimd.collective_compute(
        "AllGather", mybir.AluOpType.bypass, replica_groups=rg,
        ins=[wdin.opt()], outs=[wdout.opt()])

    # ---------- helpers ----------
    def bcast(scal, n, val, tag):
        """[n,1] f32 = val * scal (scal is [1,1])."""
        r = sp.tile([n, 1], F32, tag=tag)
        nc.gpsimd.partition_broadcast(r[:], scal[:])
        if val != 1.0:
            nc.vector.tensor_scalar_mul(r[:], r[:], float(val))
        return r

    # ---------- phase B+C: load, quantize, PE-transpose, GEMM1 ----------
    qsc = 15.0 / gmax
    ps_a1 = psb.tile([HID, SHARD], F32, tag="big")
    r = hp.tile([HID, SHARD], F32, tag="h")
    sqh = hp.tile([HID, SHARD], F32, tag="sqh")
    st_s = sp.tile([HID, NQ], F32, tag="st_s")
    st_q = sp.tile([HID, NQ], F32, tag="st_q")
    st_m = sp.tile([HID, NQ], F32, tag="st_m")

    QSIZES = (4, 4, 4, 3, 1)
    qbase = 0
    for q in range(NQ):
        qn = QSIZES[q]
        qts = qtsp.tile([128, NK * qn * 128], BF16, tag="qts")
        # layout [p, (t k c)]: tile-major -> contiguous PSUM->SBUF copies
        qtsv = qts[:].rearrange("p (t k c) -> p t k c", t=qn, k=NK)
        for tq in range(qn):
            t = qbase + tq
            st = sigp.tile([128, D_IN], F32, tag="sig")
            eng = nc.scalar if t % 2 == 0 else nc.sync
            eng.dma_start(st[:], sig[t * 128:(t + 1) * 128, :])
            v1 = tmpp.tile([128, D_IN], F32, tag="v1")
            if t % 2 == 0:
                nc.scalar.activation(v1[:], st[:],
                                     mybir.ActivationFunctionType.Copy,
                                     bias=MAGIC, scale=qsc)
            else:
                nc.vector.tensor_scalar(v1[:], st[:], qsc, MAGIC,
                                        mybir.AluOpType.mult,
                                        mybir.AluOpType.add)
            qq = qp.tile([128, KP], BF16, tag="q")
            nc.vector.memset(qq[:, D_IN:], 0.0)
            nc.vector.tensor_scalar_sub(qq[:, :D_IN], v1[:], MAGIC)
            for hh in range(2):
                ps = ptr.tile([128, KP // 2], BF16, tag=f"tr{hh}")
                for k8 in range(NK // 2):
                    k = hh * (NK // 2) + k8
                    nc.tensor.transpose(
                        ps[:, k8 * 128:(k8 + 1) * 128],
                        qq[:, k * 128:(k + 1) * 128], idb[:])
                dst = qts[:, tq * KP + hh * (KP // 2):
                          tq * KP + (hh + 1) * (KP // 2)]
                if hh == 0:
                    nc.scalar.activation(
                        dst, ps[:], mybir.ActivationFunctionType.Copy,
                        bias=0.0, scale=1.0)
                else:
                    nc.vector.tensor_copy(dst, ps[:])
        if q == 0:
            for k in range(NK):
                nc.sync.dma_start(w1c[k][:], w1t[k * 128:(k + 1) * 128, :])
        cols = slice(qbase * 128, (qbase + qn) * 128)
        qbase += qn
        for k in range(NK):
            nc.tensor.matmul(ps_a1[:, cols], w1c[k][:],
                             qtsv[:, :, k, :],
                             start=(k == 0), stop=(k == NK - 1))
        # relu+bias then per-quarter feature stats
        nc.scalar.activation(r[:, cols], ps_a1[:, cols],
                             mybir.ActivationFunctionType.Relu,
                             bias=b1_sb[:], scale=1.0)
        nc.vector.reduce_sum(st_s[:, q:q + 1], r[:, cols],
                             axis=mybir.AxisListType.X)
        nc.scalar.activation(sqh[:, cols], r[:, cols],
                             mybir.ActivationFunctionType.Square,
                             accum_out=st_q[:, q:q + 1])
        nc.vector.reduce_max(st_m[:, q:q + 1], r[:, cols],
                             axis=mybir.AxisListType.X)
        if q == 0:
            # preload the SQRT activation table off the critical path
            dum = sp.tile([1, 1], F32, tag="dum")
            nc.scalar.activation(dum[:], one1[:],
                                 mybir.ActivationFunctionType.Sqrt)

    # ---------- final local stats + one AllGather ----------
    stat3 = sp.tile([HID, 3], F32, tag="stat3")
    nc.vector.reduce_sum(stat3[:, 0:1], st_s[:], axis=mybir.AxisListType.X)
    nc.vector.reduce_sum(stat3[:, 1:2], st_q[:], axis=mybir.AxisListType.X)
    nc.vector.reduce_max(stat3[:, 2:3], st_m[:], axis=mybir.AxisListType.X)

    din = dcc.tile([HID, 3], F32, tag="di_ag")
    dout = dcc.tile([NCORES * HID, 3], F32, tag="do_ag")
    nc.gpsimd.dma_start(din[:], stat3[:])
    nc.gpsimd.collective_compute(
        "AllGather", mybir.AluOpType.bypass, replica_groups=rg,
        ins=[din.opt()], outs=[dout.opt()])
    g8 = sp.tile([HID, NCORES * 3], F32, tag="g8")
    nc.sync.dma_start(
        g8[:].rearrange("p (c s) -> p c s", c=NCORES),
        dout[:].rearrange("(c p) s -> p c s", p=HID))
    # strided reduces across the 8 core-chunks
    g8v = g8[:].rearrange("p (c s) -> p s c", c=NCORES)
    sums = sp.tile([HID, 2], F32, tag="sums")
    nc.vector.reduce_sum(sums[:], g8v[:, 0:2, :], axis=mybir.AxisListType.X)
    rmaxf = sp.tile([HID, 1], F32, tag="rmaxf")
    nc.vector.reduce_max(rmaxf[:], g8v[:, 2:3, :], axis=mybir.AxisListType.X)
    # collapse per-feature max -> global maxr
    ps_m = pss.tile([1, HID], F32, tag="psm")
    nc.tensor.transpose(ps_m[:], rmaxf[:], id_sb[:HID, :HID])
    maxr = sp.tile([1, 1], F32, tag="maxr")
    nc.vector.reduce_max(maxr[:], ps_m[:], axis=mybir.AxisListType.X)

    # ---------- BN affine folded into linear2 coefficients ----------
    rrm = sp.tile([1, 1], F32, tag="rrm")
    nc.vector.reciprocal(rrm[:], maxr[:])
    qsc2 = bcast(rrm, HID, 127.0, "qsc2")         # [HID,1] = 127/maxr
    # quantize r per 1024-chunk (Scalar; parallel to Vector fold chain below)
    for n in range(2):
        cn = slice(n * 1024, (n + 1) * 1024)
        nc.scalar.activation(r[:, cn], r[:, cn],
                             mybir.ActivationFunctionType.Copy,
                             bias=MAGIC, scale=qsc2[:])

    m12 = sp.tile([HID, 2], F32, tag="m12")
    nc.vector.tensor_scalar_mul(m12[:], sums[:], 1.0 / BATCH)
    mu2 = sp.tile([HID, 1], F32, tag="mu2")
    nc.scalar.square(mu2[:], m12[:, 0:1])
    varr = sp.tile([HID, 1], F32, tag="varr")
    nc.vector.tensor_tensor(varr[:], m12[:, 1:2], mu2[:],
                            mybir.AluOpType.subtract)
    # rounding variance of q2 (in r-units, scaled by S1^2) + eps, as the
    # sqrt bias: sd = sqrt(varr*S1^2 + (maxr*S1/(127*sqrt(12)))^2 + eps)
    rv = sp.tile([1, 1], F32, tag="rv")
    nc.scalar.activation(rv[:], maxr[:], mybir.ActivationFunctionType.Square,
                         scale=S1 / (127.0 * np.sqrt(12.0)))
    nc.vector.tensor_scalar_add(rv[:], rv[:], BN_EPS)
    rvb = bcast(rv, HID, 1.0, "rvb")
    sd = sp.tile([HID, 1], F32, tag="sd")
    nc.scalar.activation(sd[:], varr[:], mybir.ActivationFunctionType.Sqrt,
                         bias=rvb[:], scale=S1 * S1)
    isd = sp.tile([HID, 1], F32, tag="isd")
    nc.vector.reciprocal(isd[:], sd[:])
    abn = sp.tile([HID, 1], F32, tag="abn")
    nc.vector.tensor_tensor(abn[:], gam_sb[:], isd[:], mybir.AluOpType.mult)
    mu = sp.tile([HID, 1], F32, tag="mu")
    nc.vector.tensor_scalar_mul(mu[:], m12[:, 0:1], S1)
    amu = sp.tile([HID, 1], F32, tag="amu")
    nc.vector.tensor_tensor(amu[:], abn[:], mu[:], mybir.AluOpType.mult)
    cbn = sp.tile([HID, 1], F32, tag="cbn")
    nc.vector.tensor_tensor(cbn[:], bet_sb[:], amu[:],
                            mybir.AluOpType.subtract)
    # abns = abn*s2*w2s ;  y = (q2 ints) * abns  (f32, exact products)
    s2t = sp.tile([1, 1], F32, tag="s2t")
    nc.vector.tensor_scalar_mul(s2t[:], maxr[:], S1 / 127.0)
    s2b = bcast(s2t, HID, 1.0, "s2b")
    abns = sp.tile([HID, 1], F32, tag="abns")
    nc.vector.tensor_scalar(abns[:], abn[:], s2b[:], w2s,
                            mybir.AluOpType.mult, mybir.AluOpType.mult)
    y = hp.tile([HID, SHARD], F32, tag="sqh")
    # zbias[1,2] = w2s*(cbn @ w2int) + b2i*(w2s*s2)
    ps_zb = pss.tile([1, OUT], F32, tag="psm")
    nc.tensor.matmul(ps_zb[:], cbn[:], w2f[:], start=True, stop=True)
    zb1 = sp.tile([1, OUT], F32, tag="zb1")
    nc.vector.tensor_scalar_mul(zb1[:], ps_zb[:], w2s)
    rs2 = sp.tile([1, 1], F32, tag="rs2")
    nc.vector.reciprocal(rs2[:], s2t[:])
    b2sc = sp.tile([1, 1], F32, tag="b2sc")
    nc.vector.tensor_scalar_mul(b2sc[:], rs2[:], 1.0 / w2s)  # 1/(w2s*s2)
    t3 = sp.tile([1, OUT], F32, tag="t3")
    nc.scalar.activation(t3[:], b2_sb[:], mybir.ActivationFunctionType.Copy,
                         bias=MAGIC, scale=b2sc[:])
    b2i = sp.tile([1, OUT], F32, tag="b2i")
    nc.vector.tensor_scalar(b2i[:], t3[:], MAGIC, 1.0,
                            mybir.AluOpType.subtract, mybir.AluOpType.min)
    nc.vector.tensor_scalar_max(b2i[:], b2i[:], -2.0)
    b2is = sp.tile([1, OUT], F32, tag="b2is")
    nc.vector.tensor_scalar(b2is[:], b2i[:], s2t[:], w2s,
                            mybir.AluOpType.mult, mybir.AluOpType.mult)
    zbias = sp.tile([1, OUT], F32, tag="zbias")
    nc.vector.tensor_tensor(zbias[:], zb1[:], b2is[:], mybir.AluOpType.add)
    ps_zbt = pss.tile([OUT, 1], F32, tag="psm")
    nc.tensor.transpose(ps_zbt[:], zbias[:], one1[:])
    zb2 = sp.tile([OUT, 1], F32, tag="zb2")
    nc.vector.tensor_copy(zb2[:], ps_zbt[:])

    # ---------- y + GEMM2 (f32) + relu + max, pipelined per 512-chunk ------
    ps_z = psb.tile([OUT, SHARD], F32, tag="big")
    zr = zp.tile([OUT, SHARD], F32, tag="z")
    zmx = sp.tile([OUT, 2], F32, tag="zmx")
    for n in range(2):
        cn = slice(n * 1024, (n + 1) * 1024)
        nc.vector.tensor_scalar(y[:, cn], r[:, cn], MAGIC, abns[:],
                                mybir.AluOpType.subtract,
                                mybir.AluOpType.mult)
        for m in range(2):
            cm = slice(n * 1024 + m * 512, n * 1024 + (m + 1) * 512)
            nc.tensor.matmul(ps_z[:, cm], w2f[:], y[:, cm],
                             start=True, stop=True)
        if n == 0:
            nc.scalar.activation(zr[:, cn], ps_z[:, cn],
                                 mybir.ActivationFunctionType.Relu,
                                 bias=zb2[:], scale=1.0)
        else:
            nc.vector.tensor_scalar(zr[:, cn], ps_z[:, cn], zb2[:], 0.0,
                                    mybir.AluOpType.add,
                                    mybir.AluOpType.max)
        nc.vector.reduce_max(zmx[:, n:n + 1], zr[:, cn],
                             axis=mybir.AxisListType.X)
    zm2 = sp.tile([OUT, 1], F32, tag="zm2")
    nc.vector.reduce_max(zm2[:], zmx[:], axis=mybir.AxisListType.X)

    din2 = dcc.tile([OUT, 1], F32, tag="di_ag2")
    dout2 = dcc.tile([NCORES * OUT, 1], F32, tag="do_ag2")
    nc.gpsimd.dma_start(din2[:], zm2[:])
    nc.gpsimd.collective_compute(
        "AllGather", mybir.AluOpType.bypass, replica_groups=rg,
        ins=[din2.opt()], outs=[dout2.opt()])
    g4r = sp.tile([1, NCORES * OUT], F32, tag="g4r")
    nc.sync.dma_start(g4r[:], dout2[:].rearrange("a b -> b a"))
    gmz = sp.tile([1, 1], F32, tag="gmz")
    nc.vector.reduce_max(gmz[:], g4r[:], axis=mybir.AxisListType.X)

    # ---------- final quant + store ----------
    HS = SHARD // 2
    rmz = sp.tile([1, 1], F32, tag="rmz")
    nc.vector.reciprocal(rmz[:], gmz[:])
    qsc3 = bcast(rmz, OUT, 127.0, "qsc3")          # [2,1] 127/maxz
    s3b = bcast(gmz, OUT, 1.0 / 127.0, "s3b")      # [2,1] maxz/127
    t5 = zp.tile([OUT, SHARD], F32, tag="z")
    nc.scalar.activation(t5[:, :HS], zr[:, :HS],
                         mybir.ActivationFunctionType.Copy,
                         bias=MAGIC, scale=qsc3[:])
    nc.vector.tensor_scalar(t5[:, HS:], zr[:, HS:], qsc3[:], MAGIC,
                            mybir.AluOpType.mult, mybir.AluOpType.add)
    osb = zp.tile([OUT, SHARD], F32, tag="z")
    nc.vector.tensor_scalar(osb[:, HS:], t5[:, HS:], MAGIC, s3b[:],
                            mybir.AluOpType.subtract, mybir.AluOpType.mult)
    nc.scalar.activation(osb[:, :HS], t5[:, :HS],
                         mybir.ActivationFunctionType.Copy,
                         bias=-MAGIC, scale=1.0)
    nc.vector.tensor_scalar_mul(osb[:, :HS], osb[:, :HS], s3b[:])
    nc.scalar.dma_start(out[:, HS:], osb[:, HS:])
    nc.sync.dma_start(out[:, :HS], osb[:, :HS])


def _prep(sig, W1, b1, W2, b2, gamma, beta):
    sig = np.ascontiguousarray(np.asarray(sig, dtype=np.float32))
    W1 = np.asarray(W1, dtype=np.float32)
    W2 = np.asarray(W2, dtype=np.float32)
    b1 = np.asarray(b1, dtype=np.float32)
    gmax = float(np.max(np.abs(sig)))
    w1s = float(np.max(np.abs(W1)))
    w2s = float(np.max(np.abs(W2)))
    w1i = np.clip(np.round(W1 / w1s), -2, 1).astype(np.float32)
    w2i = np.clip(np.round(W2 / w2s), -2, 1).astype(np.float32)
    b1q = np.clip(np.round(b1 * (15.0 / (w1s * gmax))), -2, 1)
    w1t = np.zeros((KP, HID), dtype=ml_dtypes.bfloat16)
    w1t[:D_IN, :] = w1i.T.astype(ml_dtypes.bfloat16)
    w2t = np.ascontiguousarray(w2i.T).astype(np.float32)
    com = {
        "w1t": w1t,
        "w2t": w2t,
        "b1i": b1q.astype(np.float32).reshape(HID, 1),
        "b2": np.ascontiguousarray(np.asarray(b2, np.float32).reshape(1, OUT)),
        "gamma": np.asarray(gamma, np.float32).reshape(HID, 1),
        "bet": np.asarray(beta, np.float32).reshape(HID, 1),
        "ident": np.eye(128, dtype=np.float32),
    }
    in_maps = []
    for c in range(NCORES):
        m = dict(com)
        m["sig"] = np.ascontiguousarray(sig[c * SHARD:(c + 1) * SHARD])
        in_maps.append(m)
    return w1s, w2s, gmax, in_maps


def kernel(sig, W1, b1, W2, b2, gamma, beta):
    w1s, w2s, gmax, in_maps = _prep(sig, W1, b1, W2, b2, gamma, beta)
    key = (round(w1s, 9), round(w2s, 9), round(gmax, 9))
    if key not in _CACHE:
        _CACHE[key] = _build(w1s, w2s, gmax)
    nc = _CACHE[key]
    trace = os.environ.get("BASS_TRACE") == "1"
    try:
        res = bass_utils.run_bass_kernel_spmd(
            nc, in_maps, core_ids=list(range(NCORES)), trace=trace)
    except ModuleNotFoundError:
        res = bass_utils.run_bass_kernel_spmd(
            nc, in_maps, core_ids=list(range(NCORES)), trace=False)
    kernel.last_results = res
    return np.concatenate([r["out"].T for r in res.results], axis=0)
